# revision 15
# baseline (speedup 1.0000x reference)
"""Quantized-weight batched linear: out[b,n,m] = sum_k deq(qweight)[n,k] * x[b,k,m].

Strategy (1-term fp8 DoubleRowSwInterleave):
  - Host: dequantize weight exactly, center it (w' = w - 0.5). Centering
    shrinks both the fp8 weight error AND the activation-quantization error
    contribution (which scales with |w'|, E[w'^2] = 1/12 vs E[w^2] = 1/3).
    Quantize w' and x to fp8 e4m3. The rank-1 offset term 0.5 * colsum_k(x)
    is added back exactly on the host.
  - Device (8 cores, data-parallel over batch B=64 -> 8 batches/core):
    DoubleRowSwInterleave fp8 matmuls contract 256 per instruction (2 planes
    x 128 partitions), so each output tile needs only 4 matmuls for K=1024 —
    half the instruction count of a bf16 kernel. PSUM fp32 -> bf16 SBUF ->
    DMA out (halves store traffic).
  - Host: out = S.astype(f32) + 0.5 * colsum  (exact correction).
  Measured rel_fro error ~1.8e-2 (gate 2e-2); PE floor 512 mm x 512cy = 109us.
"""

import numpy as np
import ml_dtypes

N = 1024   # output rows (weight rows)
K = 1024   # reduction dim
M = 1024   # columns of x per batch
NGROUP = 16
GS = K // NGROUP
B = 64
NCORES = 8
BPC = B // NCORES  # batches per core
J = K // 256       # k-chunk pairs (each SwI matmul contracts 256)
NT = N // 128      # output-row tiles
MT = M // 512      # moving free-dim tiles (one PSUM bank each)

E4M3 = ml_dtypes.float8_e4m3  # TRN fp8e4: max normal 240

_CACHE = {}
LAST_RESULT = None  # BassKernelResults of the most recent run (for profiling)


def _build_nc(bpc=BPC):
    import concourse.mybir as mybir
    import concourse.tile as tile
    from concourse import bacc

    SWI = mybir.MatmulPerfMode.DoubleRowSwInterleave
    fp8 = mybir.dt.float8e4

    nc = bacc.Bacc(None, target_bir_lowering=False, debug=False)
    # SwInterleave stationary layout, per (j, n0): per partition p the 256
    # weights stream as [A127, B127, A126, B126, ..., A0, B0] where
    # A_c = w'[256j + p, 128 n0 + c], B_c = w'[256j + 128 + p, 128 n0 + c].
    wq = nc.dram_tensor("wq", [J, 128, NT, 256], fp8, kind="ExternalInput")
    # xq[b, p, j, i, m] = e4m3(x)[b, 256j + 128i + p, m] — partition-major so
    # a whole batch loads as one contiguous-per-partition 1 MB DMA.
    xq = nc.dram_tensor("xq", [bpc, 128, J, 2, M], fp8, kind="ExternalInput")
    out = nc.dram_tensor("out", [bpc, N, M], mybir.dt.bfloat16, kind="ExternalOutput")

    with tile.TileContext(nc) as tc:
        with (
            tc.tile_pool(name="wpool", bufs=J) as wpool,
            tc.tile_pool(name="xpool", bufs=bpc) as xpool,
            tc.tile_pool(name="opool", bufs=12) as opool,
            tc.tile_pool(name="psum", bufs=8, space="PSUM") as psum_pool,
        ):
            # Startup: w (4 x 256 KB) on sync, x[0] on gpsimd, x[1] on scalar —
            # three queues in parallel so the PE ramps after ~1 MB per queue.
            # Steady-state x loads go on sync; stores alternate gpsimd/sync.
            wt = [None] * J
            xt = [None] * bpc
            queues = [nc.sync, nc.gpsimd, nc.scalar]

            def load_w(j, eng):
                w_j = wpool.tile([128, NT, 256], fp8, tag=f"w{j}", name=f"w{j}")
                eng.dma_start(out=w_j[:], in_=wq[j])
                wt[j] = w_j

            def load_x(b, eng):
                x_b = xpool.tile([128, J, 2, M], fp8, tag="x", name=f"x{b}")
                eng.dma_start(out=x_b[:], in_=xq[b])
                xt[b] = x_b

            def load_x_sliced(b, eng):
                # j-granular slice DMAs so the first matmuls only wait ~256KB
                x_b = xpool.tile([128, J, 2, M], fp8, tag="x", name=f"x{b}")
                for j in range(J):
                    eng.dma_start(out=x_b[:, j], in_=xq[b, :, j])
                xt[b] = x_b

            load_x_sliced(0, nc.gpsimd)
            load_x_sliced(1, nc.scalar)
            for j in range(J):
                load_w(j, nc.sync)
            for b in range(2, bpc):
                load_x(b, nc.sync)

            st_i = 0
            for bw in range(bpc // 2):
                bpair = (2 * bw, 2 * bw + 1)
                for n0 in range(NT):
                    ps = {}
                    for b in bpair:
                        for m0 in range(MT):
                            ps[b, m0] = psum_pool.tile(
                                [128, 512], mybir.dt.float32, tag="ps",
                                name=f"ps{b}_{n0}_{m0}",
                            )
                    last_wave = bw == bpc // 2 - 1 and n0 == NT - 1

                    def drain(b):
                        nonlocal st_i
                        ot = opool.tile(
                            [128, M], mybir.dt.bfloat16, tag="o",
                            name=f"o{b}_{n0}",
                        )
                        nc.vector.tensor_copy(ot[:, 0:512], ps[b, 0][:])
                        # scalar issues startup x DMAs; keep the first waves'
                        # drains off it so PSUM banks recycle without stalls
                        cp2 = (nc.vector.tensor_copy if bw == 0 and n0 < 4
                               else nc.scalar.copy)
                        cp2(ot[:, 512:M], ps[b, 1][:])
                        if last_wave:
                            st_eng = queues[st_i % 3]  # fan the final flush
                        else:
                            st_eng = nc.gpsimd if st_i % 2 == 0 else nc.sync
                        st_i += 1
                        st_eng.dma_start(
                            out=out[b, n0 * 128:(n0 + 1) * 128, :],
                            in_=ot[:],
                        )

                    if last_wave:
                        # b-outer, and m0-outer for the final batch: each
                        # finished chunk drains under the remaining matmuls,
                        # so only the last 128 KB store is exposed at the end.
                        b0, b1 = bpair
                        for j in range(J):
                            lhsT = wt[j][:, n0, :]
                            for m0 in range(MT):
                                rhs = xt[b0][:, j, :, m0 * 512:(m0 + 1) * 512]
                                nc.tensor.matmul(
                                    ps[b0, m0][:], lhsT, rhs,
                                    start=(j == 0), stop=(j == J - 1),
                                    perf_mode=SWI,
                                )
                        drain(b0)
                        for m0 in range(MT):
                            for j in range(J):
                                lhsT = wt[j][:, n0, :]
                                rhs = xt[b1][:, j, :, m0 * 512:(m0 + 1) * 512]
                                nc.tensor.matmul(
                                    ps[b1, m0][:], lhsT, rhs,
                                    start=(j == 0), stop=(j == J - 1),
                                    perf_mode=SWI,
                                )
                            ot1 = opool.tile(
                                [128, 512], mybir.dt.bfloat16, tag="o",
                                name=f"olast{m0}",
                            )
                            cp = nc.vector.tensor_copy if m0 == 0 else nc.scalar.copy
                            cp(ot1[:], ps[b1, m0][:])
                            queues[m0].dma_start(
                                out=out[b1, n0 * 128:(n0 + 1) * 128,
                                        m0 * 512:(m0 + 1) * 512],
                                in_=ot1[:],
                            )
                    else:
                        for j in range(J):
                            lhsT = wt[j][:, n0, :]
                            for b in bpair:
                                for m0 in range(MT):
                                    rhs = xt[b][:, j, :, m0 * 512:(m0 + 1) * 512]
                                    nc.tensor.matmul(
                                        ps[b, m0][:], lhsT, rhs,
                                        start=(j == 0), stop=(j == J - 1),
                                        perf_mode=SWI,
                                    )
                        for b in bpair:
                            drain(b)
    nc.compile()
    return nc


def _prep_weights(qweight, qrange, qmin):
    # Matches reference: w = q * qrange + qmin per (row, group), fp32.
    q = np.asarray(qweight).astype(np.float32).reshape(N, NGROUP, GS)
    qr = np.asarray(qrange).astype(np.float32).reshape(N, NGROUP, 1)
    qm = np.asarray(qmin).astype(np.float32).reshape(N, NGROUP, 1)
    w = (q * qr + qm).reshape(N, K)
    wpT = np.ascontiguousarray((w - np.float32(0.5)).T)        # (K, N) fp32
    wq8 = wpT.astype(E4M3)                                     # (K, N)
    # [j, i, p, n0, c] with c reversed, then interleave i within each n0 block
    a = wq8.reshape(J, 2, 128, NT, 128)[..., ::-1]             # [j, i, p, n0, c]
    return np.ascontiguousarray(
        a.transpose(0, 2, 3, 4, 1).reshape(J, 128, NT, 256))   # [j, p, n0, 256]


def _prep_x(x):
    x = np.asarray(x, dtype=np.float32)                        # (B, K, M)
    xh8 = x.astype(E4M3)
    # (B, J, 2, 128, M) -> (B, 128, J, 2, M): partition-major per batch
    xq = np.ascontiguousarray(
        xh8.reshape(B, J, 2, 128, M).transpose(0, 3, 1, 2, 4))
    colsum = x.sum(axis=1, dtype=np.float64).astype(np.float32)  # (B, M)
    return xq, colsum


def _ensure_axon_hooks():
    """run_bass_kernel_spmd(trace=True) imports antenv.axon_hooks, which some
    images lack; provide a stub (and register the real NTFF hook if the boot
    package is present) so tracing degrades gracefully instead of crashing."""
    try:
        import antenv.axon_hooks  # noqa: F401
        return
    except ImportError:
        pass
    try:
        import sys
        import types

        import antenv

        mod = types.ModuleType("antenv.axon_hooks")
        mod._hook = None
        mod.set_axon_ntff_profile_hook = lambda h: setattr(mod, "_hook", h)
        mod.get_axon_ntff_profile_hook = lambda: mod._hook
        sys.modules["antenv.axon_hooks"] = mod
        antenv.axon_hooks = mod
        try:
            from trn_agent_boot.trn_boot import _ntff_profile_via_ctypes

            mod._hook = _ntff_profile_via_ctypes("/opt/axon/libaxon_pjrt.so")
        except Exception:
            pass
    except Exception:
        pass


def kernel(x, qweight, qrange, qmin):
    global LAST_RESULT
    _ensure_axon_hooks()
    from concourse.bass_utils import run_bass_kernel_spmd

    wq_host = _prep_weights(qweight, qrange, qmin)
    xq_host, colsum = _prep_x(x)

    if "nc" not in _CACHE:
        _CACHE["nc"] = _build_nc()
    nc = _CACHE["nc"]

    in_maps = [
        {"wq": wq_host, "xq": np.ascontiguousarray(xq_host[c * BPC:(c + 1) * BPC])}
        for c in range(NCORES)
    ]
    LAST_RESULT = run_bass_kernel_spmd(nc, in_maps, core_ids=list(range(NCORES)))
    outs = [r["out"] for r in LAST_RESULT.results]
    S = np.concatenate(outs, axis=0).astype(np.float32)        # (B, N, M)
    S += np.float32(0.5) * colsum[:, None, :]
    return S
